# revision 18
# baseline (speedup 1.0000x reference)
"""Trainium2 Bass kernel for the cross-attention module.

Sharding: core c in 0..7 handles batch b = c//2 and heads [4*(c%2), 4*(c%2)+4).
Each core computes its batch's shared projections (q conv stack, spatial gate,
channel gate) plus the kv projection slice for its 4 heads, then runs
flash-style attention for its 4 (batch, head) pairs and writes a
(2048, 256) slice of the output.

Device layout notes:
  - Everything channel-major: (C, N) with channels on SBUF partitions.
  - Attention is computed in transposed form: S^T blocks (keys x queries),
    exp on ACT with no max subtraction (scores are small; verified), then
    out^T = V_aug^T @ P^T accumulated on PE, where V_aug carries an extra
    ones column producing the softmax denominator for free.
  - GroupNorm group stats are produced with a block-diagonal ones matmul
    (group-sum + broadcast back to partitions in one PE op).  Conv biases
    are never added to activations; they are folded into the GN affine.
"""

import sys

sys.path.insert(0, "/opt/trn_rl_repo")

from contextlib import ExitStack

import numpy as np

import concourse.bacc as bacc
import concourse.bass as bass
import concourse.mybir as mybir
import concourse.tile as tile

F32 = mybir.dt.float32
AF = mybir.ActivationFunctionType
ALU = mybir.AluOpType

B, N, C = 4, 2048, 512
H, D = 8, 64
GROUPS = 32
SCALE = D ** -0.5
EPS = 1e-5
NCH = 4          # 128-channel chunks in C
FC = 4           # 512-token free chunks in N

# dtype used for matmul streaming (float32 = exact/slow, float32r = fast)
import os as _os

MM_DT = {
    "float32": mybir.dt.float32,
    "float32r": mybir.dt.float32r,
    "bfloat16": mybir.dt.bfloat16,
}[_os.environ.get("KERNEL_MM_DT", "bfloat16")]


DTM = F32 if MM_DT is F32 else MM_DT


def _mm(ap):
    return ap


# --------------------------------------------------------------------------
# Host-side input preparation (numpy only)
# --------------------------------------------------------------------------

def _prep_core_inputs(inp, core):
    b = core // 2
    half = core % 2
    f32 = np.float32

    def a(x):
        return np.ascontiguousarray(np.asarray(x, dtype=f32))

    x1t = a(inp["x1"][b]).T                       # (512, 2048)
    x1s = np.zeros((C, N + 2), dtype=f32)
    x1s[:, 1 : N + 1] = x1t
    x2t = a(inp["x2"][b]).T                       # (512, 2048)

    qc1_w = a(inp["qc1_w"])                       # (512, 16, 3)
    w1bd = np.zeros((3, NCH, 128, 128), dtype=f32)
    for t in range(3):
        for ch in range(NCH):
            for g in range(8):
                co0 = 128 * ch + 16 * g
                # block[i, j] = qc1_w[co0 + j, i, t]
                w1bd[t, ch, 16 * g : 16 * g + 16, 16 * g : 16 * g + 16] = (
                    qc1_w[co0 : co0 + 16, :, t].T
                )

    qco0 = 256 * half
    qc2_w = a(inp["qc2_w"])[:, :, 0]              # (512, 512)
    w2T = np.zeros((NCH, 2, 128, 128), dtype=f32)
    for ci in range(NCH):
        for co in range(2):
            w2T[ci, co] = qc2_w[
                qco0 + 128 * co : qco0 + 128 * co + 128,
                128 * ci : 128 * ci + 128,
            ].T

    # kv: chunk order [k0, k1, v0, v1] of 128 out-channels each
    kvc_w = a(inp["kvc_w"])[:, :, 0]              # (1024, 16)
    kv_co0 = [256 * half, 256 * half + 128, 512 + 256 * half, 512 + 256 * half + 128]
    # input rows for chunk c: global ci in [co0/2, co0/2+64) -> x2g row (ci - base)
    x2g = np.stack(
        [
            x2t[128 * half : 128 * half + 128],       # ci rows for k0 (0:64) & k1 (64:128)
            x2t[256 + 128 * half : 256 + 128 * half + 128],  # rows for v0 & v1
        ]
    )                                              # (2, 128, 2048)
    wkvp = np.zeros((4, 128, 128), dtype=f32)
    for c in range(4):
        co0 = kv_co0[c]
        row0 = 64 * (c % 2)  # offset within the stacked 128-row rhs tile
        for j in range(128):
            co = co0 + j
            g = co // 32
            ci0 = 16 * g                           # global input row base for this group
            local = ci0 - (co0 // 2)               # 0..48 within the 64 rows of this chunk
            wkvp[c, row0 + local : row0 + local + 16, j] = kvc_w[co, :]

    sg1_w = a(inp["sg1_w"])[:, :, 0]              # (128, 512)
    wsgT = np.zeros((NCH, 128, 128), dtype=f32)
    for ci in range(NCH):
        wsgT[ci] = sg1_w[:, 128 * ci : 128 * ci + 128].T
    wsg2 = a(inp["sg2_w"])[0, :, 0].reshape(128, 1)
    sg2_b = a(inp["sg2_b"]).reshape(1, 1)

    cg1_w = a(inp["cg1_w"])                       # (128, 512)
    wcg1T = np.zeros((NCH, 128, 128), dtype=f32)
    for ci in range(NCH):
        wcg1T[ci] = cg1_w[:, 128 * ci : 128 * ci + 128].T / float(N)
    wcg2T = a(inp["cg2_w"]).T.copy()              # (128, 8)
    cg2_bv = a(inp["cg2_b"]).reshape(8, 1)

    selcg = np.zeros((2, 8, 128), dtype=f32)
    for j in range(2):
        for p in range(128):
            selcg[j, 4 * half + 2 * j + p // 64, p] = 1.0

    bd16 = np.zeros((128, 128), dtype=f32)
    for g in range(8):
        bd16[16 * g : 16 * g + 16, 16 * g : 16 * g + 16] = 1.0
    ident = np.eye(128, dtype=f32)

    def vec4(x, base=0):
        x = a(x)
        return np.stack(
            [x[base + 128 * i : base + 128 * i + 128].reshape(128, 1) for i in range(4)]
        )

    def vec_kv(x):
        x = a(x)
        return np.stack([x[o : o + 128].reshape(128, 1) for o in kv_co0])

    return {
        "x1s": x1s,
        "x2g": x2g,
        "w1bd": w1bd,
        "w2T": w2T,
        "wkvp": wkvp,
        "wsgT": wsgT,
        "wsg2": wsg2,
        "sg2_b": sg2_b,
        "wcg1T": wcg1T,
        "wcg2T": wcg2T,
        "cg2_bv": cg2_bv,
        "selcg": selcg,
        "bd16": bd16,
        "ident": ident,
        "qc1_bv": vec4(inp["qc1_b"]),
        "qgn_gv": vec4(inp["qgn_g"]),
        "qgn_bv": vec4(inp["qgn_b"]),
        "qc2_bv": np.stack(
            [a(inp["qc2_b"])[qco0 + 128 * i : qco0 + 128 * i + 128].reshape(128, 1) for i in range(2)]
        ),
        "kvc_bv": vec_kv(inp["kvc_b"]),
        "kvgn_gv": vec_kv(inp["kvgn_g"]),
        "kvgn_bv": vec_kv(inp["kvgn_b"]),
        "sg1_bv": a(inp["sg1_b"]).reshape(128, 1),
        "sggn_gv": a(inp["sggn_g"]).reshape(128, 1),
        "sggn_bv": a(inp["sggn_b"]).reshape(128, 1),
        "cg1_bv": a(inp["cg1_b"]).reshape(128, 1),
        "cggn_gv": a(inp["cggn_g"]).reshape(128, 1),
        "cggn_bv": a(inp["cggn_b"]).reshape(128, 1),
    }


_INPUT_SPECS = {
    "x1s": (C, N + 2),
    "x2g": (2, 128, N),
    "w1bd": (3, NCH, 128, 128),
    "w2T": (NCH, 2, 128, 128),
    "wkvp": (4, 128, 128),
    "wsgT": (NCH, 128, 128),
    "wsg2": (128, 1),
    "sg2_b": (1, 1),
    "wcg1T": (NCH, 128, 128),
    "wcg2T": (128, 8),
    "cg2_bv": (8, 1),
    "selcg": (2, 8, 128),
    "bd16": (128, 128),
    "ident": (128, 128),
    "qc1_bv": (NCH, 128, 1),
    "qgn_gv": (NCH, 128, 1),
    "qgn_bv": (NCH, 128, 1),
    "qc2_bv": (2, 128, 1),
    "kvc_bv": (4, 128, 1),
    "kvgn_gv": (4, 128, 1),
    "kvgn_bv": (4, 128, 1),
    "sg1_bv": (128, 1),
    "sggn_gv": (128, 1),
    "sggn_bv": (128, 1),
    "cg1_bv": (128, 1),
    "cggn_gv": (128, 1),
    "cggn_bv": (128, 1),
}


# --------------------------------------------------------------------------
# Device program (shared across all 8 cores)
# --------------------------------------------------------------------------

def _group_norm_vectors(nc, pool, psum_stats, eps_ap, gamma, beta, inv_n, extra_bias):
    """From group-summed [sum, sumsq] (128,2) psum produce the per-partition
    affine (svec, bvec) implementing y = (x + cb - m) * rstd * gamma + beta,
    where cb is the folded conv bias (extra_bias, may be None) and the input
    sums already include cb.  nbias: unused. Returns (svec, bvec) SBUF APs."""
    nm = pool.tile([128, 1], F32, tag="gnsc")
    nc.vector.tensor_scalar_mul(nm[:], psum_stats[:, 0:1], -inv_n)      # -mean
    m2 = pool.tile([128, 1], F32, tag="gnsc")
    nc.vector.tensor_mul(m2[:], nm[:], nm[:])                           # mean^2
    vv = pool.tile([128, 1], F32, tag="gnsc")
    # var = sumsq/n - mean^2
    nc.vector.tensor_scalar(vv[:], psum_stats[:, 1:2], inv_n, None, ALU.mult)
    nc.vector.tensor_sub(vv[:], vv[:], m2[:])
    sd = pool.tile([128, 1], F32, tag="gnsc")
    nc.scalar.activation(sd[:], vv[:], AF.Sqrt, bias=eps_ap[:])         # sqrt(var+eps)
    rstd = pool.tile([128, 1], F32, tag="gnsc")
    nc.vector.reciprocal(rstd[:], sd[:])
    svec = pool.tile([128, 1], F32, tag="gnsv")
    nc.vector.tensor_mul(svec[:], rstd[:], gamma[:])
    bvec = pool.tile([128, 1], F32, tag="gnsv")
    if extra_bias is not None:
        tb = pool.tile([128, 1], F32, tag="gnsc")
        nc.vector.tensor_add(tb[:], nm[:], extra_bias[:])               # cb - mean
        nc.vector.scalar_tensor_tensor(
            bvec[:], tb[:], 1.0, svec[:], ALU.mult, ALU.mult
        )
    else:
        nc.vector.tensor_mul(bvec[:], nm[:], svec[:])
    nc.vector.tensor_add(bvec[:], bvec[:], beta[:])
    return svec, bvec


def _stats_to_sums(nc, pool, stats6, cbias, n_tot):
    """bn-stats blocks (128, k, 6) -> (128,2) [sum, sumsq] SBUF tile, with the
    per-partition conv bias cbias (or None) folded into the values."""
    mv = pool.tile([128, 2], F32, tag="gnmv")
    nc.vector.bn_aggr(mv[:], stats6[:])                                 # [mean, var]
    mean = pool.tile([128, 1], F32, tag="gnsc")
    if cbias is not None:
        nc.vector.tensor_add(mean[:], mv[:, 0:1], cbias[:])
    else:
        nc.vector.tensor_copy(mean[:], mv[:, 0:1])
    s2 = pool.tile([128, 2], F32, tag="gns2")
    nc.vector.tensor_scalar_mul(s2[:, 0:1], mean[:], float(n_tot))
    m2 = pool.tile([128, 1], F32, tag="gnsc")
    nc.vector.tensor_mul(m2[:], mean[:], mean[:])
    nc.vector.tensor_add(m2[:], m2[:], mv[:, 1:2])                      # var + mean^2
    nc.vector.tensor_scalar_mul(s2[:, 1:2], m2[:], float(n_tot))
    return s2


def build_program():
    nc = bacc.Bacc("TRN2", target_bir_lowering=False, debug=False)

    dram = {}
    _DTM_INPUTS = {"x1s", "x2g", "w1bd", "w2T", "wkvp", "wsgT", "wsg2"}
    for name, shape in _INPUT_SPECS.items():
        dt = DTM if name in _DTM_INPUTS else F32
        dram[name] = nc.dram_tensor(name, list(shape), dt, kind="ExternalInput")
    out_d = nc.dram_tensor("out", [N, 256], F32, kind="ExternalOutput")

    with tile.TileContext(nc) as tc, ExitStack() as ctx:
        # long-lived pools
        kt_pool = ctx.enter_context(tc.tile_pool(name="kt", bufs=1))
        qt_pool = ctx.enter_context(tc.tile_pool(name="qt", bufs=1))
        va_pool = ctx.enter_context(tc.tile_pool(name="va", bufs=1))
        small = ctx.enter_context(tc.tile_pool(name="small", bufs=4))
        vecs = ctx.enter_context(tc.tile_pool(name="vecs", bufs=1))

        stg = ctx.enter_context(tc.tile_pool(name="stage", bufs=4))

        def staged_load(pool, dram_ap, name):
            """DMA -> staging tile -> DVE copy, so matmul consumers wait only
            on the DVE semaphore (walrus caps sync-waits on LDW+MM pairs)."""
            st = stg.tile(list(dram_ap.shape), dram_ap.dtype, tag=f"st_{name}",
                          name=f"st_{name}", bufs=1)
            nc.gpsimd.dma_start(st[:], dram_ap)
            dst = pool.tile(list(dram_ap.shape), dram_ap.dtype, name=name, tag=name)
            nc.vector.tensor_copy(dst[:], st[:])
            return dst

        ident = staged_load(vecs, dram["ident"][:, :], "ident")
        KT = [kt_pool.tile([128, N], DTM, tag=f"kt{j}", name=f"kt{j}") for j in range(2)]
        QT = [qt_pool.tile([128, N], DTM, tag=f"qt{j}", name=f"qt{j}") for j in range(2)]
        VA = [va_pool.tile([128, 16 * 65], DTM, tag=f"va{h}", name=f"va{h}") for h in range(4)]

        with ExitStack() as pctx:
            # ---------------- projection phase pools ----------------
            wpool = pctx.enter_context(tc.tile_pool(name="wts", bufs=1))
            x1pool = pctx.enter_context(tc.tile_pool(name="x1", bufs=1))
            actp = pctx.enter_context(tc.tile_pool(name="acts", bufs=1))
            pst = pctx.enter_context(tc.tile_pool(name="pst", bufs=6, space="PSUM"))
            psq = pctx.enter_context(tc.tile_pool(name="psq", bufs=2, space="PSUM"))

            x1s = [x1pool.tile([128, N + 2], DTM, tag=f"x1s{ch}", name=f"x1s{ch}") for ch in range(NCH)]
            for ch in range(NCH):
                stx = stg.tile([128, N + 2], DTM, tag=f"stx1_{ch}", name=f"stx1_{ch}", bufs=1)
                nc.gpsimd.dma_start(stx[:], dram["x1s"][128 * ch : 128 * ch + 128, :])
                nc.vector.tensor_copy(x1s[ch][:], stx[:])

            bd16 = staged_load(vecs, dram["bd16"][:, :], "bd16")

            def loadv(name, idx=None):
                ap = dram[name] if idx is None else dram[name][idx]
                nm = name if idx is None else f"{name}_{idx}"
                return vecs.tile_from(ap[:, :] if idx is None else ap, name=nm)

            eps_t = vecs.tile([128, 1], F32, name="eps_t")
            nc.vector.memset(eps_t[:], EPS)
            qc1_bv = [loadv("qc1_bv", ch) for ch in range(NCH)]
            qgn_gv = [loadv("qgn_gv", ch) for ch in range(NCH)]
            qgn_bv = [loadv("qgn_bv", ch) for ch in range(NCH)]
            qc2_bv = [loadv("qc2_bv", i) for i in range(2)]
            kvc_bv = [loadv("kvc_bv", i) for i in range(4)]
            kvgn_gv = [loadv("kvgn_gv", i) for i in range(4)]
            kvgn_bv = [loadv("kvgn_bv", i) for i in range(4)]
            sg1_bv = loadv("sg1_bv")
            sggn_gv = loadv("sggn_gv")
            sggn_bv = loadv("sggn_bv")
            cg1_bv = loadv("cg1_bv")
            cggn_gv = loadv("cggn_gv")
            cggn_bv = loadv("cggn_bv")
            cg2_bv = vecs.tile_from(dram["cg2_bv"][:, :], name="cg2_bv")
            sg2_b = vecs.tile_from(dram["sg2_b"][:, :], name="sg2_bt")

            # ---------------- channel gate ----------------
            with nc.named_scope("cgate"):
                colsum = [small.tile([128, 1], F32, tag="colsum", name=f"colsum{_}") for _ in range(NCH)]
                for ch in range(NCH):
                    nc.vector.reduce_sum(
                        colsum[ch][:], x1s[ch][:, 1 : N + 1], axis=mybir.AxisListType.X
                    )
                wcg1T = [staged_load(wpool, dram["wcg1T"][ci], f"wcg1T{ci}") for ci in range(NCH)]
                ps_cg = psq.tile([128, 1], F32, tag="psq")
                for ci in range(NCH):
                    nc.tensor.matmul(
                        ps_cg[:], _mm(wcg1T[ci][:]), _mm(colsum[ci][:]),
                        start=(ci == 0), stop=(ci == NCH - 1),
                    )
                cgx = small.tile([128, 1], F32, tag="cgx")
                nc.vector.tensor_add(cgx[:], ps_cg[:], cg1_bv[:])
                st2 = small.tile([128, 2], F32, tag="cgst")
                nc.vector.tensor_copy(st2[:, 0:1], cgx[:])
                nc.vector.tensor_mul(st2[:, 1:2], cgx[:], cgx[:])
                ps_gs = psq.tile([128, 2], F32, tag="psq")
                nc.tensor.matmul(ps_gs[:], _mm(bd16[:]), _mm(st2[:]))
                svec, bvec = _group_norm_vectors(
                    nc, small, ps_gs, eps_t, cggn_gv, cggn_bv, 1.0 / 16, None
                )
                cgn = small.tile([128, 1], F32, tag="cgn")
                nc.scalar.activation(cgn[:], cgx[:], AF.Relu, bias=bvec[:], scale=svec[:])
                wcg2T = staged_load(wpool, dram["wcg2T"][:, :], "wcg2T")
                ps_c2 = psq.tile([8, 1], F32, tag="psq")
                nc.tensor.matmul(ps_c2[:], _mm(wcg2T[:]), _mm(cgn[:]))
                cgs = small.tile([8, 1], F32, tag="cgs")
                nc.scalar.activation(cgs[:], ps_c2[:], AF.Sigmoid, bias=cg2_bv[:])
                selcg = [staged_load(wpool, dram["selcg"][j], f"selcg{j}") for j in range(2)]
                cgvS = []
                for j in range(2):
                    ps_cv = psq.tile([128, 1], F32, tag="psq")
                    nc.tensor.matmul(ps_cv[:], _mm(selcg[j][:]), _mm(cgs[:]))
                    t = small.tile([128, 1], F32, tag=f"cgv{j}")
                    nc.vector.tensor_scalar_mul(t[:], ps_cv[:], SCALE)
                    cgvS.append(t)

            # ---------------- spatial gate ----------------
            with nc.named_scope("sgate"):
                wsgT = [staged_load(wpool, dram["wsgT"][ci], f"wsgT{ci}") for ci in range(NCH)]
                ps_sg = [pst.tile([128, 512], F32, tag="pp", name=f"pssg{_}") for _ in range(FC)]
                stats6 = small.tile([128, FC, 6], F32, tag="sgst6")
                for fc in range(FC):
                    for ci in range(NCH):
                        nc.tensor.matmul(
                            ps_sg[fc][:],
                            _mm(wsgT[ci][:]),
                            _mm(x1s[ci][:, 1 + 512 * fc : 1 + 512 * fc + 512]),
                            start=(ci == 0), stop=(ci == NCH - 1),
                        )
                    nc.vector.bn_stats(stats6[:, fc, :], ps_sg[fc][:])
                s2 = _stats_to_sums(nc, small, stats6, sg1_bv, N)
                ps_gs = psq.tile([128, 2], F32, tag="psq")
                nc.tensor.matmul(ps_gs[:], _mm(bd16[:]), _mm(s2[:]))
                svec, bvec = _group_norm_vectors(
                    nc, small, ps_gs, eps_t, sggn_gv, sggn_bv, 1.0 / (16 * N), sg1_bv
                )
                sgn = actp.tile([128, N], DTM, tag="sgn")
                for fc in range(FC):
                    nc.scalar.activation(
                        sgn[:, 512 * fc : 512 * fc + 512], ps_sg[fc][:],
                        AF.Relu, bias=bvec[:], scale=svec[:],
                    )
                wsg2 = staged_load(wpool, dram["wsg2"][:, :], "wsg2")
                sgrow = small.tile([1, N], F32, tag="sgrow", bufs=1)
                for fc in range(FC):
                    ps_r = psq.tile([1, 512], F32, tag="psq")
                    nc.tensor.matmul(
                        ps_r[:], _mm(wsg2[:]), _mm(sgn[:, 512 * fc : 512 * fc + 512])
                    )
                    nc.scalar.activation(
                        sgrow[:, 512 * fc : 512 * fc + 512], ps_r[:],
                        AF.Sigmoid, bias=sg2_b[:],
                    )

            # ---------------- G2 gate grid ----------------
            with nc.named_scope("g2"):
                onesr = small.tile([1, 128], F32, tag="onesr", bufs=1)
                nc.vector.memset(onesr[:], 1.0)
                G2 = [actp.tile([128, N], F32, tag=f"g2_{j}", name=f"g2_{j}") for j in range(2)]
                for fc in range(FC):
                    ps_gb = pst.tile([128, 512], F32, tag="pp")
                    nc.tensor.matmul(
                        ps_gb[:], _mm(onesr[:]), _mm(sgrow[:, 512 * fc : 512 * fc + 512])
                    )
                    for j in range(2):
                        nc.vector.tensor_scalar_mul(
                            G2[j][:, 512 * fc : 512 * fc + 512], ps_gb[:], cgvS[j][:]
                        )

            # ---------------- q conv1 + GroupNorm ----------------
            with nc.named_scope("qconv1"):
                w1bd = [
                    [staged_load(wpool, dram["w1bd"][t, ch], f"w1bd{t}_{ch}") for t in range(3)]
                    for ch in range(NCH)
                ]
                q1n = [actp.tile([128, N], DTM, tag=f"q1n{ch}", name=f"q1n{ch}") for ch in range(NCH)]
                for ch in range(NCH):
                    ps_q1 = [pst.tile([128, 512], F32, tag="pp", name=f"psq1_{_}") for _ in range(FC)]
                    stats6 = small.tile([128, FC, 6], F32, tag="q1st6")
                    for fc in range(FC):
                        for t in range(3):
                            nc.tensor.matmul(
                                ps_q1[fc][:],
                                _mm(w1bd[ch][t][:]),
                                _mm(x1s[ch][:, 512 * fc + t : 512 * fc + t + 512]),
                                start=(t == 0), stop=(t == 2),
                            )
                        nc.vector.bn_stats(stats6[:, fc, :], ps_q1[fc][:])
                    s2 = _stats_to_sums(nc, small, stats6, qc1_bv[ch], N)
                    ps_gs = psq.tile([128, 2], F32, tag="psq")
                    nc.tensor.matmul(ps_gs[:], _mm(bd16[:]), _mm(s2[:]))
                    svec, bvec = _group_norm_vectors(
                        nc, small, ps_gs, eps_t, qgn_gv[ch], qgn_bv[ch],
                        1.0 / (16 * N), qc1_bv[ch],
                    )
                    for fc in range(FC):
                        nc.vector.tensor_scalar(
                            q1n[ch][:, 512 * fc : 512 * fc + 512], ps_q1[fc][:],
                            svec[:], bvec[:], ALU.mult, ALU.add,
                        )

            # ---------------- qc2 + gates -> Q' ----------------
            with nc.named_scope("qc2"):
                w2T = [
                    [staged_load(wpool, dram["w2T"][ci, co], f"w2T{ci}_{co}") for co in range(2)]
                    for ci in range(NCH)
                ]
                for co in range(2):
                    for fc in range(FC):
                        ps_q2 = pst.tile([128, 512], F32, tag="pp")
                        for ci in range(NCH):
                            nc.tensor.matmul(
                                ps_q2[:],
                                _mm(w2T[ci][co][:]),
                                _mm(q1n[ci][:, 512 * fc : 512 * fc + 512]),
                                start=(ci == 0), stop=(ci == NCH - 1),
                            )
                        # QT = (psum + b2) * G2
                        nc.vector.scalar_tensor_tensor(
                            QT[co][:, 512 * fc : 512 * fc + 512],
                            ps_q2[:], qc2_bv[co][:],
                            G2[co][:, 512 * fc : 512 * fc + 512],
                            ALU.add, ALU.mult,
                        )

        with ExitStack() as kctx:
            wpool2 = kctx.enter_context(tc.tile_pool(name="wts2", bufs=1))
            kvio = kctx.enter_context(tc.tile_pool(name="kvio", bufs=4))
            pst2 = kctx.enter_context(tc.tile_pool(name="pst2", bufs=6, space="PSUM"))
            psq2 = kctx.enter_context(tc.tile_pool(name="psq2", bufs=2, space="PSUM"))
            # ---------------- kv projection ----------------
            with nc.named_scope("kv"):
                wkvp = [staged_load(wpool2, dram["wkvp"][c], f"wkvp{c}") for c in range(4)]
                x2g = [kvio.tile([128, N], DTM, tag="kvbig", name=f"x2g{_}") for _ in range(2)]
                for i in range(2):
                    stx = stg.tile([128, N], DTM, tag=f"stx2_{i}", name=f"stx2_{i}", bufs=1)
                    nc.gpsimd.dma_start(stx[:], dram["x2g"][i])
                    nc.vector.tensor_copy(x2g[i][:], stx[:])
                vraw = []
                for c in range(4):
                    ps_kv = [pst2.tile([128, 512], F32, tag="pp2", name=f"pskv{_}") for _ in range(FC)]
                    stats6 = small.tile([128, FC, 6], F32, tag="kvst6")
                    for fc in range(FC):
                        nc.tensor.matmul(
                            ps_kv[fc][:],
                            _mm(wkvp[c][:]),
                            _mm(x2g[c // 2][:, 512 * fc : 512 * fc + 512]),
                        )
                        nc.vector.bn_stats(stats6[:, fc, :], ps_kv[fc][:])
                    s2 = _stats_to_sums(nc, small, stats6, kvc_bv[c], N)
                    ps_gs = psq2.tile([128, 2], F32, tag="psq2k")
                    nc.tensor.matmul(ps_gs[:], _mm(bd16[:]), _mm(s2[:]))
                    svec, bvec = _group_norm_vectors(
                        nc, small, ps_gs, eps_t, kvgn_gv[c], kvgn_bv[c],
                        1.0 / (16 * N), kvc_bv[c],
                    )
                    if c < 2:
                        dst = KT[c]
                    else:
                        dst = kvio.tile([128, N], F32, tag="kvbig", name=f"vraw{c}")
                        vraw.append(dst)
                    for fc in range(FC):
                        nc.scalar.activation(
                            dst[:, 512 * fc : 512 * fc + 512], ps_kv[fc][:],
                            AF.Gelu, bias=bvec[:], scale=svec[:],
                        )

                # V transposes -> V_aug (ones col for softmax denominator)
                for h in range(4):
                    nc.vector.memset(VA[h][:], 1.0)
                for vc in range(2):
                    for j in range(16):
                        ps_vt = psq2.tile([128, 128], F32, tag="psq2k")
                        nc.tensor.transpose(
                            ps_vt[:], vraw[vc][:, 128 * j : 128 * j + 128], ident[:]
                        )
                        va0 = VA[2 * vc].rearrange("p (j w) -> p j w", w=65)
                        va1 = VA[2 * vc + 1].rearrange("p (j w) -> p j w", w=65)
                        nc.vector.tensor_copy(va0[:, j, 0:64], ps_vt[:, 0:64])
                        nc.vector.tensor_copy(va1[:, j, 0:64], ps_vt[:, 64:128])


        # ---------------- attention ----------------
        with ExitStack() as actx:
            ps_st = actx.enter_context(tc.tile_pool(name="ps_st", bufs=2, space="PSUM"))
            ps_o = actx.enter_context(tc.tile_pool(name="ps_o", bufs=1, space="PSUM"))
            ps_tr = actx.enter_context(tc.tile_pool(name="ps_tr", bufs=2, space="PSUM"))
            ptp = actx.enter_context(tc.tile_pool(name="ptp", bufs=2))
            otp = actx.enter_context(tc.tile_pool(name="otp", bufs=2))
            obp = actx.enter_context(tc.tile_pool(name="obp", bufs=4))

            with nc.named_scope("attn"):
                for h in range(4):
                    ch, hl = h // 2, h % 2
                    kt = KT[ch]
                    qt = QT[ch]
                    va = VA[h].rearrange("p (j w) -> p j w", w=65)
                    for ih in range(2):
                        i0 = 1024 * ih
                        po = ps_o.tile([65, 1024], F32, tag="po")
                        for j in range(16):
                            ps = ps_st.tile([128, 1024], F32, tag="pst")
                            for s in range(2):
                                nc.tensor.matmul(
                                    ps[:, 512 * s : 512 * s + 512],
                                    _mm(kt[64 * hl : 64 * hl + 64, 128 * j : 128 * j + 128]),
                                    _mm(qt[64 * hl : 64 * hl + 64, i0 + 512 * s : i0 + 512 * s + 512]),
                                )
                            pt = ptp.tile([128, 1024], DTM, tag="pt")
                            nc.scalar.activation(pt[:], ps[:], AF.Exp)
                            for s in range(2):
                                nc.tensor.matmul(
                                    po[:, 512 * s : 512 * s + 512],
                                    _mm(va[:, j, :]),
                                    _mm(pt[:, 512 * s : 512 * s + 512]),
                                    start=(j == 0), stop=(j == 15),
                                )
                        ot = otp.tile([65, 1024], F32, tag="ot")
                        nc.vector.tensor_copy(ot[:], po[:])
                        for ib in range(8):
                            ps_t = ps_tr.tile([128, 65], F32, tag="ps_t")
                            nc.tensor.transpose(
                                ps_t[:], ot[:, 128 * ib : 128 * ib + 128], ident[0:65, 0:65]
                            )
                            rvec = small.tile([128, 1], F32, tag="rvec")
                            nc.vector.reciprocal(rvec[:], ps_t[:, 64:65])
                            ob = obp.tile([128, 64], F32, tag="ob")
                            nc.vector.tensor_scalar_mul(ob[:], ps_t[:, 0:64], rvec[:])
                            r0 = i0 + 128 * ib
                            nc.sync.dma_start(
                                out_d[r0 : r0 + 128, 64 * h : 64 * h + 64], ob[:]
                            )
    nc.compile()
    return nc


# --------------------------------------------------------------------------
# Entry point
# --------------------------------------------------------------------------

TRACE = False
LAST_RESULTS = None


def _cast_in_maps(in_maps):
    if MM_DT is F32:
        return in_maps
    import ml_dtypes
    np_dt = {"bfloat16": ml_dtypes.bfloat16, "float32r": np.float32}[MM_DT.name]
    heavy = {"x1s", "x2g", "w1bd", "w2T", "wkvp", "wsgT", "wsg2"}
    return [
        {k: (v.astype(np_dt) if k in heavy else v) for k, v in m.items()}
        for m in in_maps
    ]


def kernel(**inputs):
    global LAST_RESULTS
    from concourse.bass_utils import run_bass_kernel_spmd

    core_ids = list(range(8))
    in_maps = _cast_in_maps([_prep_core_inputs(inputs, c) for c in core_ids])
    nc = build_program()
    res = run_bass_kernel_spmd(nc, in_maps, core_ids, trace=TRACE)
    LAST_RESULTS = res
    out = np.zeros((B, N, C), dtype=np.float32)
    for c in core_ids:
        b, half = c // 2, c % 2
        out[b, :, 256 * half : 256 * half + 256] = res.results[c]["out"]
    return out


if __name__ == "__main__":
    import reference

    inputs = {k: np.asarray(v) for k, v in reference.setup_inputs().items()}
    out = kernel(**inputs)
    print("kernel out", out.shape, out.dtype)
